# revision 24
# baseline (speedup 1.0000x reference)
"""Causal self-attention (B=2, T=2048, C=1024, H=16) on 8 Trainium2 cores.

Sharding: tensor-parallel over heads (2 heads/core). Each core computes
QKV projection for its heads, causal attention, and a partial c_proj
output; partials are summed on the host (b_proj and the v-bias
contribution bv@W_proj are added on the host).

Numerics / dataflow:
  - QKV projection runs in fp8e4 DoubleRow mode (2 contraction tiles per
    instruction, 0.5 PE cycles/row) with hi+lo error compensation:
    x ~ x_hi + x_lo, W*WS ~ W_hi + W_lo (WS=1024 scale centers W in the
    e4m3 range), and PSUM accumulates x_hi@W_hi + x_hi@W_lo + x_lo@W_hi.
    The WS scale is folded into the exp() scale (q,k paths) and into the
    softmax-sum ones column (v path), so nothing ever rescales on-chip.
  - qT/kT/et/vp/ynT/wp are bf16: same PE matmul rate as fp32r at width
    >=256 but no 4x penalty on narrow diagonal tiles, 2x DVE mode for
    the causal-mask multiply, and half the DMA/SBUF footprint.
  - v' is computed directly in [t, d] layout (lhsT = x8 tile), removing
    the PE transposes; the AV lhsT is then a plain PSUM->SBUF copy.
  - S^T tiles for adjacent q-tiles share a 2-bank PSUM region so one
    exp() covers both (ACT is the secondary bottleneck).
  - A fill queue threads QKV chains and c_proj column tiles into the
    ACT-bound attention streams so the PE never idles at phase ends.
"""

from collections import deque

import numpy as np
import ml_dtypes

import concourse.bass as bass
import concourse.tile as tile
from concourse import bacc, mybir
from concourse.bass_utils import run_bass_kernel_spmd

F32 = mybir.dt.float32
BF16 = mybir.dt.bfloat16
FP8 = mybir.dt.float8e4
E4NP = ml_dtypes.float8_e4m3
BFNP = ml_dtypes.bfloat16

B, T, C, H = 2, 2048, 1024, 16
HS = C // H            # 64 head dim
NCORES = 8
HL = H // NCORES       # 2 local heads
LC = HL * HS           # 128 local q/k/v cols
R = B * T              # 4096 rows (b, t)
KCP = C // 256         # 4 DoubleRow contraction chunks (256 each)
QT = 512               # attention q tile
NQT = T // QT          # 4
KA = 128               # attention k chunk (partition dim)
RT = 512               # row tile for projections
NRT = R // RT          # 8
NCC = C // 128         # 8 c_proj output chunks
WS = 1024.0            # fp8 weight scale
AVLAG = 2              # AV runs this many k chunks behind the exp
REV_LAST = True        # b1h1 runs pair (2,3) before pair (0,1)
DR = mybir.MatmulPerfMode.DoubleRow


def build_program():
    nc = bacc.Bacc("TRN2", target_bir_lowering=False, debug=False,
                   num_devices=NCORES)

    x8 = nc.dram_tensor("x8", [128, KCP, 2, 2, R], FP8, kind="ExternalInput").ap()
    wqk8 = nc.dram_tensor("wqk8", [128, 2, KCP, 2, 2 * LC], FP8,
                          kind="ExternalInput").ap()
    wv8 = nc.dram_tensor("wv8", [128, 2, KCP, 2, LC], FP8,
                         kind="ExternalInput").ap()
    bqk = nc.dram_tensor("bqk", [128, 2], F32, kind="ExternalInput").ap()
    wp = nc.dram_tensor("wp", [LC, C], BF16, kind="ExternalInput").ap()
    ident = nc.dram_tensor("ident", [KA, KA], BF16, kind="ExternalInput").ap()
    mneg = nc.dram_tensor("mneg", [KA, KA], BF16, kind="ExternalInput").ap()
    outT = nc.dram_tensor("outT", [C, R], BF16, kind="ExternalOutput").ap()
    outT_r = outT.rearrange("(cc p) r -> p cc r", p=128)

    with tile.TileContext(nc) as tc:
        with (
            tc.tile_pool(name="consts", bufs=1) as consts,
            tc.tile_pool(name="weights", bufs=1) as weights,
            tc.tile_pool(name="qkvT", bufs=1) as qkvT_pool,
            tc.tile_pool(name="xs", bufs=3) as xs_pool,
            tc.tile_pool(name="vp", bufs=32) as vp_pool,
            tc.tile_pool(name="et", bufs=8) as et_pool,
            tc.tile_pool(name="rec", bufs=2) as rec_pool,
            tc.tile_pool(name="bcs", bufs=2) as bcs_pool,
            tc.tile_pool(name="dscr", bufs=4, space="DRAM") as dscr_pool,
            tc.tile_pool(name="osb", bufs=3) as osb_pool,
            tc.tile_pool(name="mm512", bufs=2, space="PSUM") as mm512_pool,
            tc.tile_pool(name="spair", bufs=2, space="PSUM") as spair_pool,
            tc.tile_pool(name="ytps", bufs=2, space="PSUM") as ytps_pool,
        ):
            # ---- constants ----
            ones64 = consts.tile([1, HS], BF16)
            nc.vector.memset(ones64, 1.0)
            ident_sb = consts.tile([KA, KA], BF16)
            mneg_sb = consts.tile([KA, KA], BF16)
            bqk_sb = consts.tile([128, 2], F32)

            wqk_sb = weights.tile([128, 2, KCP, 2, 2 * LC], FP8)
            wv_sb = weights.tile([128, 2, KCP, 2, LC], FP8)
            wp_sb = weights.tile([LC, C], BF16)

            # first chunks land fast so matmuls can start early
            nc.sync.dma_start(out=wqk_sb[:, 0:1, 0:1], in_=wqk8[:, 0:1, 0:1])
            nc.sync.dma_start(out=wqk_sb[:, 1:2, 0:1], in_=wqk8[:, 1:2, 0:1])

            def load_consts():
                nc.sync.dma_start(out=wqk_sb[:, :, 1:KCP], in_=wqk8[:, :, 1:KCP])
                nc.sync.dma_start(out=wv_sb, in_=wv8)
                nc.sync.dma_start(out=bqk_sb, in_=bqk)

            def load_consts2():
                nc.sync.dma_start(out=ident_sb, in_=ident)
                nc.sync.dma_start(out=mneg_sb, in_=mneg)
                nc.sync.dma_start(out=wp_sb, in_=wp)

            # ---- QKV projection (fp8 DoubleRow, hi+lo compensated) ----
            qT_s = qkvT_pool.tile([LC, R], BF16, tag="qT")
            kT_s = qkvT_pool.tile([LC, R], BF16, tag="kT")
            ynT_s = qkvT_pool.tile([LC, R], BF16, tag="ynT")
            vp_tiles = {}

            # (x, w) hi/lo set pairs: hi*hi + hi*lo + lo*hi
            SETS = ((0, 0), (0, 1), (1, 0))

            def qkv_load(rt, split=False):
                # x loads go through the Pool SWDGE queue: Pool is idle and
                # this keeps the shared HWDGE issue slot free for weights,
                # output stores and the normalize bounce
                x_sb = xs_pool.tile([128, KCP, 2, 2, RT], FP8, tag="xs",
                                    name=f"x_sb_rt{rt}")
                rsl = slice(rt * RT, (rt + 1) * RT)
                if split:
                    nc.gpsimd.dma_start(out=x_sb[:, 0:1, 0:1],
                                        in_=x8[:, 0:1, 0:1, :, rsl])
                    nc.gpsimd.dma_start(out=x_sb[:, 0:1, 1:2],
                                        in_=x8[:, 0:1, 1:2, :, rsl])
                    nc.gpsimd.dma_start(out=x_sb[:, 1:KCP],
                                        in_=x8[:, 1:KCP, :, :, rsl])
                else:
                    nc.gpsimd.dma_start(out=x_sb, in_=x8[:, :, :, :, rsl])
                return x_sb

            def qk_chain(rt, x_sb, col):
                """col 0 = q, col 1 = k: 12 DoubleRow matmuls + bias add."""
                ps = mm512_pool.tile([128, RT], F32, tag="mm512",
                                     name=f"qk_ps_rt{rt}c{col}")
                n = 0
                for kcp in range(KCP):
                    for sx, sw in SETS:
                        nc.tensor.matmul(
                            ps,
                            wqk_sb[:, sw, kcp, :, col * LC:(col + 1) * LC],
                            x_sb[:, kcp, sx],
                            start=(n == 0),
                            stop=(n == 3 * KCP - 1),
                            perf_mode=DR,
                        )
                        n += 1
                dst = qT_s if col == 0 else kT_s
                nc.vector.tensor_scalar_add(
                    dst[:, rt * RT:(rt + 1) * RT], ps, bqk_sb[:, col:col + 1])

            def v_chain(rt, x_sb, tsub):
                """v' [t 128, d 128] for one 128-row chunk, direct layout."""
                b = rt // (NRT // B)
                chunk = (rt % (NRT // B)) * (RT // KA) + tsub
                ps = mm512_pool.tile([128, LC], F32, tag="mm512",
                                     name=f"v_ps_rt{rt}t{tsub}")
                tsl = slice(tsub * KA, (tsub + 1) * KA)
                n = 0
                for kcp in range(KCP):
                    for sx, sw in SETS:
                        nc.tensor.matmul(
                            ps,
                            x_sb[:, kcp, sx, :, tsl],
                            wv_sb[:, sw, kcp],
                            start=(n == 0),
                            stop=(n == 3 * KCP - 1),
                            perf_mode=DR,
                        )
                        n += 1
                vp2 = vp_pool.tile([KA, HL, HS + 1], BF16, tag="vp",
                                   name=f"vp_b{b}c{chunk}")
                # [128, 128] PSUM -> the two per-head 64-col slices
                nc.vector.tensor_copy(
                    vp2[:, :, 0:HS],
                    bass.AP(tensor=ps.tensor, offset=ps.offset,
                            ap=[list(ps.ap[0]), [HS, HL], [1, HS]]),
                )
                # softmax-sum column carries the WS scale of vp
                nc.vector.memset(vp2[:, :, HS:HS + 1], WS)
                vp_tiles[(b, chunk)] = vp2

            def proj_cc_pair(rt, cc0, o_sb, split=False, store="half"):
                """c_proj column chunks cc0, cc0+1 of row tile rt."""
                for cc in (cc0, cc0 + 1):
                    pps = mm512_pool.tile([128, RT], F32, tag="mm512",
                                          name=f"pps_rt{rt}c{cc}")
                    nc.tensor.matmul(
                        pps,
                        wp_sb[:, cc * 128:(cc + 1) * 128],
                        ynT_s[:, rt * RT:(rt + 1) * RT],
                        start=True,
                        stop=True,
                    )
                    if split:
                        half = RT // 2
                        nc.scalar.activation(
                            o_sb[:, cc, 0:half], pps[:, 0:half],
                            mybir.ActivationFunctionType.Identity)
                        nc.vector.tensor_copy(o_sb[:, cc, half:RT],
                                              pps[:, half:RT])
                    else:
                        nc.vector.tensor_copy(o_sb[:, cc], pps)
                    if store == "cc":
                        # per-column stores: shortest possible DMA tail
                        nc.sync.dma_start(
                            out=outT_r[:, cc:cc + 1, rt * RT:(rt + 1) * RT],
                            in_=o_sb[:, cc:cc + 1])
                # half-row-tile stores keep the DMA queue short at the tail
                if store == "half" and cc0 + 2 in (NCC // 2, NCC):
                    h0 = cc0 + 2 - NCC // 2
                    nc.sync.dma_start(
                        out=outT_r[:, h0:h0 + NCC // 2,
                                   rt * RT:(rt + 1) * RT],
                        in_=o_sb[:, h0:h0 + NCC // 2])

            def proj_items(rt, split=False, store="half"):
                o_sb = osb_pool.tile([128, NCC, RT], BF16, tag="osb",
                                     name=f"o_sb_rt{rt}")
                return [(lambda rt=rt, cc0=cc0, o_sb=o_sb:
                         proj_cc_pair(rt, cc0, o_sb, split=split,
                                      store=store))
                        for cc0 in range(0, NCC, 2)]

            def proj_tail(rt):
                """Last row tile: attention PSUM pools are free, so run all
                8 column chunks into 8 PSUM slots at once, then stream the
                copies on DVE and ACT in parallel with per-pair stores."""
                o_sb = osb_pool.tile([128, NCC, RT], BF16, tag="osb",
                                     name=f"o_sb_rt{rt}")
                ps = []
                for i in range(2):
                    ps.append(mm512_pool.tile([128, RT], F32, tag="mm512",
                                              name=f"tpp{i}"))
                for i in range(2):
                    wide = spair_pool.tile([128, 2 * RT], F32, tag="sp",
                                           name=f"tps{i}")
                    ps.append(wide[:, 0:RT])
                    ps.append(wide[:, RT:2 * RT])
                for i in range(2):
                    ps.append(ytps_pool.tile([128, RT], F32, tag="yt",
                                             name=f"tpy{i}"))
                for cc in range(NCC):
                    nc.tensor.matmul(
                        ps[cc],
                        wp_sb[:, cc * 128:(cc + 1) * 128],
                        ynT_s[:, rt * RT:(rt + 1) * RT],
                        start=True,
                        stop=True,
                    )
                for cc in range(NCC):
                    half = RT // 2
                    nc.scalar.activation(
                        o_sb[:, cc, 0:half], ps[cc][:, 0:half],
                        mybir.ActivationFunctionType.Identity)
                    nc.vector.tensor_copy(o_sb[:, cc, half:RT],
                                          ps[cc][:, half:RT])
                    if cc % 2 == 1:
                        nc.sync.dma_start(
                            out=outT_r[:, cc - 1:cc + 1,
                                       rt * RT:(rt + 1) * RT],
                            in_=o_sb[:, cc - 1:cc + 1])

            fill = deque()

            def drain_one(_kc=None):
                if fill:
                    fill.popleft()()

            def drain_all():
                while fill:
                    fill.popleft()()

            EXP_SCALE = 1.0 / (np.sqrt(HS) * WS * WS)

            def attn_pair(b, h, p, qts, on_qt_done=None, drains=1):
                """Attention for q-tiles `qts` (1 or 2, sharing one PSUM S
                region and one exp per k chunk). `on_qt_done(qt)` fires
                right after the AV that completes a non-final qt so its
                normalize overlaps the remaining k chunks."""
                base = b * T
                hsl = slice(h * HS, (h + 1) * HS)
                nk = (max(qts) + 1) * (QT // KA)
                yps = {}
                for qt in qts:
                    yps[qt] = ytps_pool.tile([HS + 1, QT], F32, tag="yt",
                                             name=f"yt_b{b}h{h}q{qt}")
                W = QT * len(qts)
                backlog = []
                for kc in range(nk):
                    sp = spair_pool.tile([128, W], F32, tag="sp",
                                         name=f"sp_b{b}h{h}p{p}k{kc}")
                    et = et_pool.tile([128, W], BF16, tag="et",
                                      name=f"et_b{b}h{h}p{p}k{kc}")
                    cols = []   # (qt, half, off)
                    for i, qt in enumerate(qts):
                        if kc >= (qt + 1) * (QT // KA):
                            continue
                        off = max(0, kc * KA - qt * QT)
                        cols.append((qt, i, off))
                        ksl = slice(base + kc * KA, base + (kc + 1) * KA)
                        if kc * KA // QT == qt:
                            # diagonal block: pre-accumulate -1e12 on the
                            # strictly-lower triangle so exp() emits exact
                            # zeros (mask without a Pool op on the et path)
                            nc.tensor.matmul(
                                sp[:, i * QT + off:i * QT + off + KA],
                                ident_sb,
                                mneg_sb,
                                start=True,
                                stop=False,
                            )
                            nc.tensor.matmul(
                                sp[:, i * QT + off:i * QT + off + KA],
                                kT_s[hsl, ksl],
                                qT_s[hsl, base + qt * QT + off:
                                     base + qt * QT + off + KA],
                                start=False,
                                stop=True,
                            )
                            if off + KA < QT:
                                nc.tensor.matmul(
                                    sp[:, i * QT + off + KA:(i + 1) * QT],
                                    kT_s[hsl, ksl],
                                    qT_s[hsl, base + qt * QT + off + KA:
                                         base + (qt + 1) * QT],
                                    start=True,
                                    stop=True,
                                )
                        else:
                            nc.tensor.matmul(
                                sp[:, i * QT + off:(i + 1) * QT],
                                kT_s[hsl, ksl],
                                qT_s[hsl, base + qt * QT + off:
                                     base + (qt + 1) * QT],
                                start=True,
                                stop=True,
                            )
                    start = cols[0][1] * QT + cols[0][2]
                    nc.scalar.activation(
                        et[:, start:W], sp[:, start:W],
                        mybir.ActivationFunctionType.Exp,
                        scale=EXP_SCALE,
                    )
                    # AV lags the exp by AVLAG k chunks so PE never waits
                    # on ACT latency
                    backlog.append((kc, et, cols))
                    if len(backlog) > AVLAG:
                        done = av_step(b, h, yps, *backlog.pop(0))
                        for qt in done:
                            if qt != max(qts) and on_qt_done:
                                on_qt_done(qt, yps[qt])
                    for _ in range(drains):
                        drain_one()
                while backlog:
                    done = av_step(b, h, yps, *backlog.pop(0))
                    for qt in done:
                        if qt != max(qts) and on_qt_done:
                            on_qt_done(qt, yps[qt])
                return yps

            def av_step(b, h, yps, kc, et, cols):
                done = []
                for qt, i, off in cols:
                    last = kc == (qt + 1) * (QT // KA) - 1
                    nc.tensor.matmul(
                        yps[qt][:, off:QT],
                        vp_tiles[(b, kc)][:, h],
                        et[:, i * QT + off:(i + 1) * QT],
                        start=(kc == 0),
                        stop=last,
                    )
                    if last:
                        done.append(qt)
                return done

            def normalize(b, h, qt, yp, last=False):
                base = b * T
                hsl = slice(h * HS, (h + 1) * HS)
                rec = rec_pool.tile([1, QT], BF16, tag="rec",
                                    name=f"rec_b{b}h{h}q{qt}")
                with nc.allow_low_precision(
                        reason="bf16 softmax reciprocal: ~4e-3 rel err ok"):
                    nc.vector.reciprocal(rec, yp[HS:HS + 1, :])
                bcs = bcs_pool.tile([HS, QT], BF16, tag="bcs",
                                    name=f"bcs_b{b}h{h}q{qt}")
                if last:
                    # tail: DRAM-bounce latency would gate the final c_proj
                    bcp = mm512_pool.tile([HS, QT], F32, tag="mm512",
                                          name=f"bcp_b{b}h{h}q{qt}")
                    nc.tensor.matmul(bcp, ones64, rec, start=True, stop=True)
                    nc.vector.tensor_copy(bcs, bcp)
                else:
                    recd = dscr_pool.tile([1, QT], BF16, tag="recd",
                                          name=f"recd_b{b}h{h}q{qt}")
                    nc.sync.dma_start(out=recd, in_=rec)
                    rec_bcast = bass.AP(
                        tensor=recd.tensor, offset=recd.offset,
                        ap=[[0, HS]] + [list(d) for d in recd.ap[1:]])
                    nc.sync.dma_start(out=bcs, in_=rec_bcast)
                nc.vector.tensor_mul(
                    ynT_s[hsl, base + qt * QT: base + (qt + 1) * QT],
                    yp[0:HS, :],
                    bcs,
                )

            # ---- schedule ----
            loads = {}

            def get_x(rt):
                if rt not in loads:
                    loads[rt] = qkv_load(rt)
                return loads[rt]

            def rt_items(rt):
                def qk(col):
                    x = get_x(rt)
                    if col == 0 and rt + 1 < NRT:
                        get_x(rt + 1)   # prefetch next tile's DMA
                    qk_chain(rt, x, col)
                return ([lambda: qk(0), lambda: qk(1)]
                        + [(lambda t=t: v_chain(rt, loads[rt], t))
                           for t in range(RT // KA)])

            loads[0] = qkv_load(0, split=True)
            load_consts()
            loads[1] = qkv_load(1)
            load_consts2()
            loads[2] = qkv_load(2)

            # rt0/rt1 computed up front; rt2..3 fill b0h0 pair(0,1);
            # b1 QKV fills b0h0 pair(2,3) / b0h1; b0 proj spreads over
            # b0h1 pair(2,3), b1h0 and b1h1's first q-tile; b1 proj fills
            # b1h1 (descending singles, proj one row tile behind).
            for it in rt_items(0) + rt_items(1):
                it()
            fill.extend(rt_items(2) + rt_items(3))

            def norm_cb(b, h):
                def cb(qt, yp):
                    normalize(b, h, qt, yp)
                    drain_one()
                return cb

            rt7 = rt_items(7)

            def head_pairs(b, h, pre=None, cbs=None):
                out_yps = {}
                for p, qts in enumerate(((0, 1), (2, 3))):
                    if pre and pre.get(p):
                        for add in pre[p]:
                            fill.extend(add() if callable(add) else add)
                    cb = (cbs or {}).get(p) or norm_cb(b, h)
                    yps = attn_pair(b, h, p, qts, on_qt_done=cb)
                    drain_one()
                    last = (b, h, p) == (B - 1, HL - 1, 1)
                    normalize(b, h, qt := max(qts), yps[qt], last=last)
                    drain_one()
                return out_yps

            def drain_then(*item_lists):
                def go():
                    drain_all()
                    out = []
                    for il in item_lists:
                        out.extend(il() if callable(il) else il)
                    return out
                return go

            # b0h0: pair(0,1) <- rt2,rt3 (hard drain: pair(2,3) reads them);
            # pair(2,3) <- b1 QKV rt4,rt5
            head_pairs(0, 0, pre={
                0: [lambda: rt_items(2) + rt_items(3)],
                1: [drain_then(lambda: rt_items(4) + rt_items(5))],
            })

            # b0h1: p01 <- rt6; p23 <- rt7 qk + proj rt0,rt1 (norms 0,1 done)
            head_pairs(0, 1, pre={
                0: [lambda: rt_items(6)],
                1: [lambda: rt7[:2] + proj_items(0) + proj_items(1)],
            })
            drain_all()

            # b1h0: p01 <- rt7 v-chains + proj rt2; p23 <- proj rt3 half
            p3 = proj_items(3)
            head_pairs(1, 0, pre={
                0: [lambda: rt7[2:] + proj_items(2)],
                1: [lambda: p3[:2]],
            })

            # b1h1: long pair (2,3) FIRST so its norms release proj rt6/rt7
            # as fill for the short pair (0,1); the tail is then only the
            # qt1 norm + proj rt5.
            b, h = B - 1, HL - 1

            def cb23(qt, yp):
                normalize(b, h, qt, yp)
                drain_one()
                if qt == 2:
                    fill.extend(proj_items(b * NQT + 2))

            def cb01(qt, yp):
                normalize(b, h, qt, yp)
                drain_one()
                if qt == 0:
                    fill.extend(proj_items(b * NQT + 0))

            if REV_LAST:
                fill.extend(p3[2:])
                yps = attn_pair(b, h, 20, (2, 3), on_qt_done=cb23)
                drain_one()
                normalize(b, h, 3, yps[3])
                drain_one()
                fill.extend(proj_items(b * NQT + 3))
                yps = attn_pair(b, h, 21, (0, 1), on_qt_done=cb01, drains=2)
                drain_one()
                normalize(b, h, 1, yps[1], last=True)
                drain_all()
                proj_tail(b * NQT + 1)
            else:
                fill.extend(p3[2:])
                yps = attn_pair(b, h, 20, (0, 1), on_qt_done=cb01)
                drain_one()
                normalize(b, h, 1, yps[1])
                drain_one()
                fill.extend(proj_items(b * NQT + 1))
                yps = attn_pair(b, h, 21, (2, 3), on_qt_done=cb23)
                drain_one()
                normalize(b, h, 3, yps[3], last=True)
                drain_all()
                proj_tail(b * NQT + 3)

    nc.compile()
    nc.compile()
    return nc


_NC = None


def _get_nc():
    global _NC
    if _NC is None:
        _NC = build_program()
    return _NC


def _pack_dr(a):
    """[C, M] -> [128, KCP, 2, M] with c = kcp*256 + r*128 + p."""
    Cd, M = a.shape
    return np.ascontiguousarray(
        a.reshape(KCP, 2, 128, M).transpose(2, 0, 1, 3))


def _hilo(a, axis=1):
    """fp32 -> fp8 hi/lo stacked at `axis`."""
    hi = a.astype(E4NP)
    lo = (a - hi.astype(np.float32)).astype(E4NP)
    return np.ascontiguousarray(np.stack([hi, lo], axis=axis))


def make_in_maps(x, W_attn, b_attn, W_proj, b_proj):
    x = np.asarray(x, np.float32)
    W_attn = np.asarray(W_attn, np.float32)
    b_attn = np.asarray(b_attn, np.float32)
    W_proj = np.asarray(W_proj, np.float32)

    xT = np.ascontiguousarray(x.reshape(R, C).T)
    x8 = _hilo(_pack_dr(xT), axis=2)
    identn = np.eye(KA, dtype=np.float32).astype(BFNP)
    mnegn = np.tril(np.full((KA, KA), -1e12, np.float32), -1).astype(BFNP)

    in_maps = []
    for core in range(NCORES):
        g0 = core * LC
        cols = slice(g0, g0 + LC)
        wqk = np.concatenate(
            [W_attn[:, i * C:(i + 1) * C][:, cols] for i in range(2)], axis=1)
        bqk_local = np.stack(
            [b_attn[i * C:(i + 1) * C][cols] * WS for i in range(2)], axis=1)
        in_maps.append({
            "x8": x8,
            "wqk8": _hilo(_pack_dr(wqk * WS)),
            "wv8": _hilo(_pack_dr(W_attn[:, 2 * C:3 * C][:, cols] * WS)),
            "bqk": np.ascontiguousarray(bqk_local, dtype=np.float32),
            "wp": np.ascontiguousarray(W_proj[cols, :]).astype(BFNP),
            "ident": identn,
            "mneg": mnegn,
        })
    return in_maps


def kernel(x, W_attn, b_attn, W_proj, b_proj):
    nc = _get_nc()
    in_maps = make_in_maps(x, W_attn, b_attn, W_proj, b_proj)
    res = run_bass_kernel_spmd(nc, in_maps, list(range(NCORES)))
    acc = res.results[0]["outT"].astype(np.float32)
    for corer in res.results[1:]:
        acc += corer["outT"].astype(np.float32)
    b_attn = np.asarray(b_attn, np.float32)
    b_eff = (np.asarray(b_proj, np.float32)
             + b_attn[2 * C:3 * C] @ np.asarray(W_proj, np.float32))
    acc += b_eff[:, None]
    return np.ascontiguousarray(acc.T).reshape(B, T, C)

